# revision 7
# baseline (speedup 1.0000x reference)
"""Trainium2 Bass kernel for nn_EncodingP (vq_codebook soft-assignment encoding).

Reference computation (B=4, D=256, K=32, H=W=64, N=H*W=4096):
    Xf = X.reshape(B, D, N).transpose(0, 2, 1)            # (B, N, D)
    L[b,n,k] = ||x_bn||^2 - 2 <x_bn, c_k> + ||c_k||^2     # (B, N, K)
    A = softmax(L * scale, axis=-1)                        # (B, N, K)
    E[b,k,d] = sum_n A[b,n,k] * x_bn[d] - (sum_n A[b,n,k]) * c_k[d]

Sharding: 8 cores = 4 batches x 2 halves of N; host sums the four
group-partials per core half (E is linear in the n-sum).

Per-core dataflow (fp16 phase 2, no hi/lo residual split — rel err ~7e-3
vs the 2e-2 gate):
  phase 1 (fp16 matmuls -> fp32 PSUM [128,512]; col-group j holds n-chunk j):
    psL[32j+k, nn] = -2*xc + x2     (x2 via an all-ones stationary over x^2)
  per 128-col block c, pipelined:
    exp (fp32 ACT with per-partition scale/bias; max |scale*L| ~ 79 < 88
      so no max-subtract; exp values reach e^79 so fp32 is mandatory)
    ptz = expS_blk^T [identones]: ONE fp32 matmul whose moving operand is
      [I_128 | blockones_128x4]; cols 0:128 are the A^T tile, cols 128:132
      are Z^T (the per-n softmax denominators, one col per j-group)
    rz = 1/Z (DVE, PSUM->SBUF), anorm_blk = ptz * rz broadcast -> fp16
  phase 2 (4-way col-tiled, fp16): psE4[32g+k, :] += anorm^T @ xts_t for
    tile t = 4g + c; xts is laid out c-major on the host so quarter c of
    the DMA unlocks all 4 strips of step c.
  Output: one fp32->fp16 copy of psE4[:, 0:257], DMA'd out; the host sums
  the 4 group partials and applies the -Asum*C correction (tiny).
"""

import os

import numpy as np

import concourse.bass as bass
import concourse.tile as tile
from concourse import mybir
from concourse.masks import make_identity

B, D, K, H, W = 4, 256, 32, 64, 64
N = H * W            # 4096
NCORES = 8
NSH = B * N // NCORES  # 2048 positions per core
NT = NSH // 128        # 16 n-tiles per core
NAUG = D + 1           # 257: X^T columns + ones column

F32 = mybir.dt.float32
F16 = mybir.dt.float16

# cst16 (fp16) column layout
_CT0 = 0      # [0:32)    -2*C^T for d-block 0
_CT1 = 32     # [32:64)   -2*C^T for d-block 1
_ONE = 64     # [64:96)   ones
_CF16 = 96
# cst32 (fp32) column layout
_SCL = 0      # scale_k (tiled x4 over partition groups)
_BIA = 1      # scale_k * ||c_k||^2
_BLK = 2      # [2:6) block indicator: cst32[p, 2+g] = (p//32 == g)
_CF32 = 8


def build_device_kernel(nc):
    xdn_d = nc.declare_dram_parameter("xdn", [D, NSH], F16, isOutput=False)
    xta_d = nc.declare_dram_parameter("xta", [128, NT * NAUG], F16,
                                      isOutput=False)
    c16_d = nc.declare_dram_parameter("cst16", [128, _CF16], F16, isOutput=False)
    c32_d = nc.declare_dram_parameter("cst32", [128, _CF32], F32, isOutput=False)
    bon_d = nc.declare_dram_parameter("bones", [128, 4], F32, isOutput=False)
    out_d = nc.declare_dram_parameter("eout", [128, NAUG], F16, isOutput=True)

    act = mybir.ActivationFunctionType
    alu = mybir.AluOpType
    HW_ = 8 * NAUG  # columns per xts half (phase-2 steps c=0,1 / c=2,3)

    with tile.TileContext(nc) as tc:
        with (
            tc.tile_pool(name="sb", bufs=1) as sb,
            tc.tile_pool(name="ps", bufs=1, space="PSUM") as ps,
            tc.tile_pool(name="psT", bufs=2, space="PSUM") as psT,
        ):
            cst16 = sb.tile([128, _CF16], F16)
            cst32 = sb.tile([128, _CF32], F32)
            x0 = sb.tile([128, NSH], F16)
            x1 = sb.tile([128, NSH], F16)
            sq0 = sb.tile([128, NSH], F16)
            sq1 = sb.tile([128, NSH], F16)
            xts = sb.tile([128, NT * NAUG], F16)
            idon = sb.tile([128, 132], F32)   # [I_128 | blockones]

            # bulk inputs ride the sync HWDGE ring (drains FIFO: issue
            # order = bandwidth priority); consts + output use scalar's.
            nc.sync.dma_start(out=x0[:], in_=xdn_d[0:128, :])
            nc.sync.dma_start(out=x1[:], in_=xdn_d[128:256, :])
            for q in range(2):
                nc.sync.dma_start(out=xts[:, HW_ * q:HW_ * (q + 1)],
                                  in_=xta_d[:, HW_ * q:HW_ * (q + 1)])
            nc.scalar.dma_start(out=cst16[:], in_=c16_d[:])
            nc.scalar.dma_start(out=cst32[:], in_=c32_d[:])
            nc.scalar.dma_start(out=idon[:, 128:132], in_=bon_d[:])
            # cols 0:128 built on device (memset + diagonal); cols 128:132
            # come from the blockones DMA — regions stay disjoint
            make_identity(nc, idon[:, 0:128])

            # one-wait hygiene: absorb DMA/gpsimd completions into each
            # engine's program order early (several instruction types can
            # carry only one sync wait; extra waits cost EVSEM chains).
            dummy = ps.tile([1, 128], F32, tag="dummy")
            scr = sb.tile([128, 16], F32)
            # HAM warmup: fp32 dummy matmuls on the (zeroed) idon while the
            # x0 DMA streams, so phase 1 runs at 2.4 GHz instead of 1.2.
            # make_identity's affine_select is reordered after these reads
            # (cross-engine WAR); gpsimd is idle so that costs nothing.
            for _ in range(4):
                nc.tensor.matmul(dummy[:], idon[:, 0:1], idon[:, 0:128],
                                 start=True, stop=True)
            # absorb cst16 (scalar-ring DMA) into PE program order
            nc.tensor.matmul(dummy[:, 0:16], cst16[:, 0:1], cst16[:, 0:16],
                             start=True, stop=True)
            # absorb cst32 into Scalar program order
            nc.scalar.copy(out=scr[:, 0:2], in_=cst32[:, 0:2])

            # squares on device: sq = x^2 (fp16 out, fp32 internal); the two
            # chunks of each d-block go to different engines so a d-block's
            # squares finish in one op-latency
            nc.scalar.square(out=sq0[:, 0:1024], in_=x0[:, 0:1024])
            nc.vector.tensor_mul(sq0[:, 1024:2048], x0[:, 1024:2048],
                                 x0[:, 1024:2048])
            nc.scalar.square(out=sq1[:, 0:1024], in_=x1[:, 0:1024])
            nc.vector.tensor_mul(sq1[:, 1024:2048], x1[:, 1024:2048],
                                 x1[:, 1024:2048])

            # phase 1: psL[32j+k, nn] = -2*xc + x2 for n = 512j + nn.
            # interleaved starts across partition-disjoint col groups are
            # numerically fine (per-partition pending-zero), only the sim's
            # partition-blind group check needs skipping.
            psL = ps.tile([128, 512], F32, tag="psL")
            for j in range(4):
                nc.tensor.matmul(
                    psL[32 * j:32 * (j + 1), :],
                    cst16[:, _CT0:_CT0 + 32],
                    x0[:, 512 * j:512 * (j + 1)],
                    start=True, stop=False,
                    tile_position=(0, 32 * j), skip_group_check=True,
                )
            # absorb the blockones DMA while waiting on sq0/x1
            nc.tensor.matmul(dummy[:, 0:4], idon[:, 128:129],
                             idon[:, 128:132], start=True, stop=True)
            for j in range(4):
                nc.tensor.matmul(
                    psL[32 * j:32 * (j + 1), :],
                    cst16[:, _ONE:_ONE + 32],
                    sq0[:, 512 * j:512 * (j + 1)],
                    start=False, stop=False,
                    tile_position=(0, 32 * j), skip_group_check=True,
                )
            for j in range(4):
                nc.tensor.matmul(
                    psL[32 * j:32 * (j + 1), :],
                    cst16[:, _CT1:_CT1 + 32],
                    x1[:, 512 * j:512 * (j + 1)],
                    start=False, stop=False,
                    tile_position=(0, 32 * j), skip_group_check=True,
                )
            for j in range(4):
                nc.tensor.matmul(
                    psL[32 * j:32 * (j + 1), :],
                    cst16[:, _ONE:_ONE + 32],
                    sq1[:, 512 * j:512 * (j + 1)],
                    start=False, stop=True,
                    tile_position=(0, 32 * j), skip_group_check=True,
                )

            # per-block softmax + phase-2, pipelined per 128-col block c.
            # ptz = expS_blk^T [I | blockones]: cols 0:128 = A^T tile,
            # cols 128:132 = Z^T (denominator for each j-group's n's).
            expS = sb.tile([128, 512], F32)
            rzT = sb.tile([128, 16], F32)
            anh = sb.tile([128, 512], F16)
            psE4 = ps.tile([128, 272], F32, tag="psE4")
            for c in range(4):
                blk = slice(128 * c, 128 * (c + 1))
                nc.scalar.activation(
                    out=expS[:, blk], in_=psL[:, blk], func=act.Exp,
                    bias=cst32[:, _BIA:_BIA + 1], scale=cst32[:, _SCL:_SCL + 1],
                )
                if c % 2 == 0:
                    # absorb this pair of steps' xts half into PE order
                    q = c // 2
                    nc.tensor.matmul(dummy[:, 0:16], xts[:, HW_ * q:HW_ * q + 1],
                                     xts[:, HW_ * q:HW_ * q + 16],
                                     start=True, stop=True)
                ptz = psT.tile([128, 132], F32, tag="ptz")
                nc.tensor.matmul(ptz[:], expS[:, blk], idon[:],
                                 start=True, stop=True)
                zc = slice(4 * c, 4 * (c + 1))
                nc.vector.reciprocal(rzT[:, zc], ptz[:, 128:132])
                nc.vector.tensor_tensor(
                    out=anh[:, blk].rearrange("p (g k) -> p g k", k=K),
                    in0=ptz[:, 0:128].rearrange("p (g k) -> p g k", k=K),
                    in1=rzT[:, zc].rearrange("p (g x) -> p g x", x=1).broadcast_to(
                        [128, 4, K]),
                    op=alu.mult,
                )
                for g in range(4):
                    pos = 4 * c + g
                    col = 128 * c + 32 * g
                    nc.tensor.matmul(
                        psE4[32 * g:32 * (g + 1), 0:NAUG],
                        anh[:, col:col + 32],
                        xts[:, NAUG * pos:NAUG * (pos + 1)],
                        start=(c == 0), stop=(c == 3),
                        tile_position=(0, 32 * g), skip_group_check=True,
                    )

            # evacuate the 4-group partials as fp16; the host does the
            # final 4-way group sum and the -Asum*C correction (tiny)
            full16 = sb.tile([128, NAUG], F16)
            nc.scalar.copy(out=full16[:], in_=psE4[:, 0:NAUG])
            nc.scalar.dma_start(out=out_d[:], in_=full16[:])

    return nc


def make_host_inputs(X, codewords, scale):
    """Shard + lay out inputs for the 8 cores. Returns list of in_maps."""
    X = np.ascontiguousarray(X, dtype=np.float32)
    codewords = np.asarray(codewords, dtype=np.float32)
    scale = np.asarray(scale, dtype=np.float32)

    c2 = (codewords.astype(np.float64) ** 2).sum(axis=1)
    cst16 = np.zeros((128, _CF16), dtype=np.float16)
    ctn2 = (-2.0 * codewords.T).astype(np.float16)        # [D, K]
    cst16[:, _CT0:_CT0 + K] = ctn2[0:128]
    cst16[:, _CT1:_CT1 + K] = ctn2[128:256]
    cst16[:, _ONE:_ONE + K] = 1.0
    cst32 = np.zeros((128, _CF32), dtype=np.float32)
    cst32[:, _SCL] = np.tile(scale, 4)
    cst32[:, _BIA] = np.tile((scale.astype(np.float64) * c2).astype(np.float32), 4)
    p = np.arange(128)
    bones = np.zeros((128, 4), dtype=np.float32)
    for g in range(4):
        bones[:, g] = (p // 32 == g).astype(np.float32)

    Xr = X.reshape(B, D, N)
    in_maps = []
    for core in range(NCORES):
        b, h = core // 2, core % 2
        xdn = np.ascontiguousarray(Xr[b][:, NSH * h:NSH * (h + 1)])
        xdn16 = xdn.astype(np.float16)
        xta = np.concatenate(
            [xdn16.T, np.ones((NSH, 1), dtype=np.float16)], axis=1)  # [NSH, 257]
        # device tile order is c-major: position 4c+g holds n-tile t=4g+c,
        # so DMA quarter c delivers exactly phase-2 step c's four tiles
        xta_dev = np.empty((128, NT * NAUG), dtype=np.float16)
        for t in range(NT):
            pos = 4 * (t % 4) + t // 4
            xta_dev[:, NAUG * pos:NAUG * (pos + 1)] = xta[128 * t:128 * (t + 1)]
        in_maps.append({"xdn": xdn16, "xta": np.ascontiguousarray(xta_dev),
                        "cst16": cst16, "cst32": cst32, "bones": bones})
    return in_maps


def gather_output(results, codewords):
    E = np.zeros((B, K, D), dtype=np.float32)
    for core, res in enumerate(results):
        full4 = res["eout"].astype(np.float32).reshape(4, K, NAUG)
        part = full4.sum(axis=0)                      # [K, NAUG]
        E[core // 2] += part[:, 0:D] - part[:, D:D + 1] * codewords
    return E


_NC_CACHE = {}


def _get_nc():
    if "nc" not in _NC_CACHE:
        # Bacc (not plain Bass): its compile() runs the TRN2 sync-wait
        # legalization (max 1 wait per instruction) that walrus requires.
        from concourse import bacc
        nc = build_device_kernel(bacc.Bacc(None))
        if not nc.is_finalized():
            nc.finalize()  # Bacc.finalize = compile (wait legalization) + freeze
        _NC_CACHE["nc"] = nc
    return _NC_CACHE["nc"]


def _install_ntff_hook_shim():
    """Fabricate antenv.axon_hooks if the image lacks it (profiling only)."""
    import sys
    import types
    try:
        from antenv.axon_hooks import get_axon_ntff_profile_hook  # noqa: F401
        return
    except ImportError:
        pass
    from trn_agent_boot.trn_boot import _ntff_profile_via_ctypes
    hook = _ntff_profile_via_ctypes("/opt/axon/libaxon_pjrt.so")
    mod = types.ModuleType("antenv.axon_hooks")
    mod._hook = hook
    mod.get_axon_ntff_profile_hook = lambda: mod._hook
    mod.set_axon_ntff_profile_hook = lambda h: setattr(mod, "_hook", h)
    sys.modules["antenv.axon_hooks"] = mod
    import antenv
    antenv.axon_hooks = mod


def kernel(X, codewords, scale):
    from concourse.bass_utils import run_bass_kernel_spmd

    nc = _get_nc()
    in_maps = make_host_inputs(X, codewords, scale)
    trace = bool(int(os.environ.get("VQ_KERNEL_TRACE", "0")))
    kwargs = {}
    if trace:
        try:
            _install_ntff_hook_shim()
            tmpdir = os.environ.get("VQ_KERNEL_TMPDIR")
            if tmpdir:
                os.makedirs(tmpdir, exist_ok=True)
                kwargs["tmpdir"] = tmpdir
        except Exception as e:  # profiling must never break execution
            print(f"ntff hook install failed: {e}")
            trace = False
    res = run_bass_kernel_spmd(nc, in_maps, core_ids=list(range(NCORES)),
                               trace=trace, **kwargs)
    if trace and res.exec_time_ns is not None:
        print(f"HW exec time: {res.exec_time_ns} ns")
    return gather_output(res.results, np.asarray(codewords, np.float32))


# revision 19
# speedup vs baseline: 1.0583x; 1.0583x over previous
"""Trainium2 Bass kernel for nn_EncodingP (vq_codebook soft-assignment encoding).

Reference computation (B=4, D=256, K=32, H=W=64, N=H*W=4096):
    Xf = X.reshape(B, D, N).transpose(0, 2, 1)            # (B, N, D)
    L[b,n,k] = ||x_bn||^2 - 2 <x_bn, c_k> + ||c_k||^2     # (B, N, K)
    A = softmax(L * scale, axis=-1)                        # (B, N, K)
    E[b,k,d] = sum_n A[b,n,k] * x_bn[d] - (sum_n A[b,n,k]) * c_k[d]

Sharding: 8 cores = 4 batches x 2 halves of N; host sums the four
group-partials per core half (E is linear in the n-sum).

Per-core dataflow (fp16 phase 2, no hi/lo residual split — rel err ~7e-3
vs the 2e-2 gate):
  phase 1 (fp16 matmuls -> fp32 PSUM [128,512]; col-group j holds n-chunk j):
    psL[32j+k, nn] = -2*xc + x2     (x2 via an all-ones stationary over x^2)
  per 128-col block c, pipelined:
    exp (fp32 ACT with per-partition scale/bias; max |scale*L| ~ 79 < 88
      so no max-subtract; exp values reach e^79 so fp32 is mandatory)
    ptz = expS_blk^T [identones]: ONE fp32 matmul whose moving operand is
      [I_128 | blockones_128x4]; cols 0:128 are the A^T tile, cols 128:132
      are Z^T (the per-n softmax denominators, one col per j-group)
    rz = 1/Z (DVE, PSUM->SBUF), anorm_blk = ptz * rz broadcast -> fp16
  phase 2 (4-way col-tiled, fp16): psE4[32g+k, :] += anorm^T @ xts_t for
    tile t = 4g + c; xts is laid out c-major on the host so quarter c of
    the DMA unlocks all 4 strips of step c.
  Output: one fp32->fp16 copy of psE4[:, 0:257], DMA'd out; the host sums
  the 4 group partials and applies the -Asum*C correction (tiny).
"""

import os

import numpy as np

import concourse.bass as bass
import concourse.tile as tile
from concourse import mybir
from concourse.masks import make_identity

B, D, K, H, W = 4, 256, 32, 64, 64
N = H * W            # 4096
NCORES = 8
NSH = B * N // NCORES  # 2048 positions per core
NT = NSH // 128        # 16 n-tiles per core
NAUG = D + 1           # 257: X^T columns + ones column

F32 = mybir.dt.float32
F16 = mybir.dt.float16
U8 = mybir.dt.uint8

# cst16 (fp16) column layout
_CT0 = 0      # [0:32)    -2*C^T for d-block 0
_CT1 = 32     # [32:64)   -2*C^T for d-block 1
_ONE = 64     # [64:96)   ones
_CF16 = 96
# cst32 (fp32) column layout
_SCL = 0      # scale_k (tiled x4 over partition groups)
_BIA = 1      # scale_k * ||c_k||^2
_CF32 = 8
# packed const tile: cst16 (96 f16) | cst32 (8 f32) | bones (4 f32)
_CALL = _CF16 + 2 * _CF32 + 8   # 120 f16 columns = 240 B/partition


def build_device_kernel(nc):
    xdn_d = nc.declare_dram_parameter("xdn", [D, NSH], F16, isOutput=False)
    xta_d = nc.declare_dram_parameter("xta", [128, NT * NAUG], F16,
                                      isOutput=False)
    cst_d = nc.declare_dram_parameter("cst", [128, 2 * _CALL], U8, isOutput=False)
    out_d = nc.declare_dram_parameter("eout", [128, NAUG], F16, isOutput=True)

    act = mybir.ActivationFunctionType
    alu = mybir.AluOpType
    HW_ = 8 * NAUG  # columns per xts half (phase-2 steps c=0,1 / c=2,3)

    with tile.TileContext(nc) as tc:
        with (
            tc.tile_pool(name="sb", bufs=1) as sb,
            tc.tile_pool(name="ps", bufs=1, space="PSUM") as ps,
            tc.tile_pool(name="psT", bufs=2, space="PSUM") as psT,
        ):
            cst = sb.tile([128, 2 * _CALL], U8)  # packed: cst16 | cst32 | bones
            cst16 = cst[:, 0:2 * _CF16].bitcast(F16)
            cst32 = cst[:, 2 * _CF16:2 * _CF16 + 4 * _CF32].bitcast(F32)
            bones = cst[:, 2 * _CF16 + 4 * _CF32:2 * _CALL].bitcast(F32)
            x0 = sb.tile([128, NSH], F16)
            x1 = sb.tile([128, NSH], F16)
            sq0 = sb.tile([128, NSH], F16)
            sq1 = sb.tile([128, NSH], F16)
            xts = sb.tile([128, NT * NAUG], F16)
            idon = sb.tile([128, 132], F32)   # [I_128 | blockones]

            # all input DMAs ride the sync HWDGE ring, which drains FIFO:
            # issue order = bandwidth priority. The consts go first (tiny),
            # and in ONE packed DMA — per-partition lines under ~200B
            # fragment into tiny packets that starve behind bulk traffic
            # when issued on the scalar ring.
            nc.sync.dma_start(out=cst[:], in_=cst_d[:])
            nc.sync.dma_start(out=x0[:], in_=xdn_d[0:128, :])
            nc.sync.dma_start(out=x1[:], in_=xdn_d[128:256, :])
            for q in range(2):
                nc.sync.dma_start(out=xts[:, HW_ * q:HW_ * (q + 1)],
                                  in_=xta_d[:, HW_ * q:HW_ * (q + 1)])
            # cols 0:128 built on device (memset + diagonal); cols 128:132
            # copied from the packed consts. All idon writers live on
            # gpsimd so consumers see a single producer engine.
            make_identity(nc, idon[:, 0:128])
            nc.gpsimd.tensor_copy(idon[:, 128:132], bones[:, 0:4])

            # one-wait hygiene: absorb DMA/gpsimd completions into each
            # engine's program order early (several instruction types can
            # carry only one sync wait; extra waits cost EVSEM chains).
            dummy = ps.tile([1, 128], F32, tag="dummy")
            scr = sb.tile([128, 16], F32)
            # HAM warmup: fp32 dummy matmuls on the identity while the
            # x0 DMA streams, so phase 1 runs at 2.4 GHz instead of 1.2
            for _ in range(4):
                nc.tensor.matmul(dummy[:], idon[:, 0:1], idon[:, 0:128],
                                 start=True, stop=True)
            # absorb the packed-const DMA into PE program order
            nc.tensor.matmul(dummy[:, 0:16], cst16[:, 0:1], cst16[:, 0:16],
                             start=True, stop=True)
            # absorb the packed-const DMA into Scalar program order
            nc.scalar.copy(out=scr[:, 0:2], in_=cst32[:, 0:2])

            # squares on device: sq = x^2 (fp16 out, fp32 internal); the two
            # chunks of each d-block go to different engines so a d-block's
            # squares finish in one op-latency
            nc.scalar.square(out=sq0[:, 0:1024], in_=x0[:, 0:1024])
            nc.vector.tensor_mul(sq0[:, 1024:2048], x0[:, 1024:2048],
                                 x0[:, 1024:2048])
            nc.scalar.square(out=sq1[:, 0:1024], in_=x1[:, 0:1024])
            nc.vector.tensor_mul(sq1[:, 1024:2048], x1[:, 1024:2048],
                                 x1[:, 1024:2048])

            # phase 1: psL[32j+k, nn] = -2*xc + x2 for n = 512j + nn.
            # interleaved starts across partition-disjoint col groups are
            # numerically fine (per-partition pending-zero), only the sim's
            # partition-blind group check needs skipping.
            psL = ps.tile([128, 512], F32, tag="psL")
            for j in range(4):
                nc.tensor.matmul(
                    psL[32 * j:32 * (j + 1), :],
                    cst16[:, _CT0:_CT0 + 32],
                    x0[:, 512 * j:512 * (j + 1)],
                    start=True, stop=False,
                    tile_position=(0, 32 * j), skip_group_check=True,
                )
            # absorb the gpsimd blockones copy while waiting on sq0/x1
            nc.tensor.matmul(dummy[:, 0:4], idon[:, 128:129],
                             idon[:, 128:132], start=True, stop=True)
            for j in range(4):
                nc.tensor.matmul(
                    psL[32 * j:32 * (j + 1), :],
                    cst16[:, _ONE:_ONE + 32],
                    sq0[:, 512 * j:512 * (j + 1)],
                    start=False, stop=False,
                    tile_position=(0, 32 * j), skip_group_check=True,
                )
            for j in range(4):
                nc.tensor.matmul(
                    psL[32 * j:32 * (j + 1), :],
                    cst16[:, _CT1:_CT1 + 32],
                    x1[:, 512 * j:512 * (j + 1)],
                    start=False, stop=False,
                    tile_position=(0, 32 * j), skip_group_check=True,
                )
            for j in range(4):
                nc.tensor.matmul(
                    psL[32 * j:32 * (j + 1), :],
                    cst16[:, _ONE:_ONE + 32],
                    sq1[:, 512 * j:512 * (j + 1)],
                    start=False, stop=True,
                    tile_position=(0, 32 * j), skip_group_check=True,
                )

            # per-block softmax + phase-2, pipelined per 128-col block c.
            # ptz = expS_blk^T [I | blockones]: cols 0:128 = A^T tile,
            # cols 128:132 = Z^T (denominator for each j-group's n's).
            expS = sb.tile([128, 512], F32)
            rzT = sb.tile([128, 16], F32)
            anh = sb.tile([128, 512], F16)
            psE4 = ps.tile([128, 272], F32, tag="psE4")
            for c in range(4):
                blk = slice(128 * c, 128 * (c + 1))
                nc.scalar.activation(
                    out=expS[:, blk], in_=psL[:, blk], func=act.Exp,
                    bias=cst32[:, _BIA:_BIA + 1], scale=cst32[:, _SCL:_SCL + 1],
                )
                if c % 2 == 0:
                    # absorb this pair of steps' xts half into PE order
                    q = c // 2
                    nc.tensor.matmul(dummy[:, 0:16], xts[:, HW_ * q:HW_ * q + 1],
                                     xts[:, HW_ * q:HW_ * q + 16],
                                     start=True, stop=True)
                ptz = psT.tile([128, 132], F32, tag="ptz")
                nc.tensor.matmul(ptz[:], expS[:, blk], idon[:],
                                 start=True, stop=True)
                zc = slice(4 * c, 4 * (c + 1))
                nc.vector.reciprocal(rzT[:, zc], ptz[:, 128:132])
                nc.vector.tensor_tensor(
                    out=anh[:, blk].rearrange("p (g k) -> p g k", k=K),
                    in0=ptz[:, 0:128].rearrange("p (g k) -> p g k", k=K),
                    in1=rzT[:, zc].rearrange("p (g x) -> p g x", x=1).broadcast_to(
                        [128, 4, K]),
                    op=alu.mult,
                )
                for g in range(4):
                    pos = 4 * c + g
                    col = 128 * c + 32 * g
                    nc.tensor.matmul(
                        psE4[32 * g:32 * (g + 1), 0:NAUG],
                        anh[:, col:col + 32],
                        xts[:, NAUG * pos:NAUG * (pos + 1)],
                        start=(c == 0), stop=(c == 3),
                        tile_position=(0, 32 * g), skip_group_check=True,
                    )

            # evacuate the 4-group partials as fp16; the host does the
            # final 4-way group sum and the -Asum*C correction (tiny)
            full16 = sb.tile([128, NAUG], F16)
            nc.scalar.copy(out=full16[:], in_=psE4[:, 0:NAUG])
            nc.scalar.dma_start(out=out_d[:], in_=full16[:])

    return nc


def make_host_inputs(X, codewords, scale):
    """Shard + lay out inputs for the 8 cores. Returns list of in_maps."""
    X = np.ascontiguousarray(X, dtype=np.float32)
    codewords = np.asarray(codewords, dtype=np.float32)
    scale = np.asarray(scale, dtype=np.float32)

    c2 = (codewords.astype(np.float64) ** 2).sum(axis=1)
    cst16 = np.zeros((128, _CF16), dtype=np.float16)
    ctn2 = (-2.0 * codewords.T).astype(np.float16)        # [D, K]
    cst16[:, _CT0:_CT0 + K] = ctn2[0:128]
    cst16[:, _CT1:_CT1 + K] = ctn2[128:256]
    cst16[:, _ONE:_ONE + K] = 1.0
    cst32 = np.zeros((128, _CF32), dtype=np.float32)
    cst32[:, _SCL] = np.tile(scale, 4)
    cst32[:, _BIA] = np.tile((scale.astype(np.float64) * c2).astype(np.float32), 4)
    p = np.arange(128)
    bones = np.zeros((128, 4), dtype=np.float32)
    for g in range(4):
        bones[:, g] = (p // 32 == g).astype(np.float32)
    cst_all = np.zeros((128, 2 * _CALL), dtype=np.uint8)
    cst_all[:, 0:2 * _CF16] = cst16.view(np.uint8)
    cst_all[:, 2 * _CF16:2 * _CF16 + 4 * _CF32] = cst32.view(np.uint8)
    cst_all[:, 2 * _CF16 + 4 * _CF32:] = bones.view(np.uint8)

    Xr = X.reshape(B, D, N)
    in_maps = []
    for core in range(NCORES):
        b, h = core // 2, core % 2
        xdn = np.ascontiguousarray(Xr[b][:, NSH * h:NSH * (h + 1)])
        xdn16 = xdn.astype(np.float16)
        xta = np.concatenate(
            [xdn16.T, np.ones((NSH, 1), dtype=np.float16)], axis=1)  # [NSH, 257]
        # device tile order is c-major: position 4c+g holds n-tile t=4g+c,
        # so DMA quarter c delivers exactly phase-2 step c's four tiles
        xta_dev = np.empty((128, NT * NAUG), dtype=np.float16)
        for t in range(NT):
            pos = 4 * (t % 4) + t // 4
            xta_dev[:, NAUG * pos:NAUG * (pos + 1)] = xta[128 * t:128 * (t + 1)]
        in_maps.append({"xdn": xdn16, "xta": np.ascontiguousarray(xta_dev),
                        "cst": cst_all})
    return in_maps


def gather_output(results, codewords):
    E = np.zeros((B, K, D), dtype=np.float32)
    for core, res in enumerate(results):
        full4 = res["eout"].astype(np.float32).reshape(4, K, NAUG)
        part = full4.sum(axis=0)                      # [K, NAUG]
        E[core // 2] += part[:, 0:D] - part[:, D:D + 1] * codewords
    return E


_NC_CACHE = {}


def _get_nc():
    if "nc" not in _NC_CACHE:
        # Bacc (not plain Bass): its compile() runs the TRN2 sync-wait
        # legalization (max 1 wait per instruction) that walrus requires.
        from concourse import bacc
        nc = build_device_kernel(bacc.Bacc(None))
        if not nc.is_finalized():
            nc.finalize()  # Bacc.finalize = compile (wait legalization) + freeze
        _NC_CACHE["nc"] = nc
    return _NC_CACHE["nc"]


def _install_ntff_hook_shim():
    """Fabricate antenv.axon_hooks if the image lacks it (profiling only)."""
    import sys
    import types
    try:
        from antenv.axon_hooks import get_axon_ntff_profile_hook  # noqa: F401
        return
    except ImportError:
        pass
    from trn_agent_boot.trn_boot import _ntff_profile_via_ctypes
    hook = _ntff_profile_via_ctypes("/opt/axon/libaxon_pjrt.so")
    mod = types.ModuleType("antenv.axon_hooks")
    mod._hook = hook
    mod.get_axon_ntff_profile_hook = lambda: mod._hook
    mod.set_axon_ntff_profile_hook = lambda h: setattr(mod, "_hook", h)
    sys.modules["antenv.axon_hooks"] = mod
    import antenv
    antenv.axon_hooks = mod


def kernel(X, codewords, scale):
    from concourse.bass_utils import run_bass_kernel_spmd

    nc = _get_nc()
    in_maps = make_host_inputs(X, codewords, scale)
    trace = bool(int(os.environ.get("VQ_KERNEL_TRACE", "0")))
    kwargs = {}
    if trace:
        try:
            _install_ntff_hook_shim()
            tmpdir = os.environ.get("VQ_KERNEL_TMPDIR")
            if tmpdir:
                os.makedirs(tmpdir, exist_ok=True)
                kwargs["tmpdir"] = tmpdir
        except Exception as e:  # profiling must never break execution
            print(f"ntff hook install failed: {e}")
            trace = False
    res = run_bass_kernel_spmd(nc, in_maps, core_ids=list(range(NCORES)),
                               trace=trace, **kwargs)
    if trace and res.exec_time_ns is not None:
        print(f"HW exec time: {res.exec_time_ns} ns")
    return gather_output(res.results, np.asarray(codewords, np.float32))
